# revision 22
# baseline (speedup 1.0000x reference)
# Causal self-attention (GQA, RoPE) on 8 NeuronCores.
#
# Sharding: sequence-parallel. Core c = (batch b = c//4, role r = c%4).
# Each batch's 2048 tokens are split into 8 chunks of 256; role r owns
# chunks {r, 7-r} (zigzag, balances causal work). Each core computes
# QKV for its 512 tokens, AllGathers RoPE'd K^T and ones-augmented V
# within its 4-core batch group, runs causal attention for a uniform
# 24-slot schedule (SPMD needs identical instruction streams; per-core
# causal validity is data: invalid slots get an additive -30000 exp
# bias so their weights are exactly zero), then projects its own
# token rows. No reduction needed after proj.
#
# v2: weight streaming in contiguous 128-col strips (PE never waits on
# HBM), proj weights prefetched to SBUF during attention, softmax
# denominators evacuated from PSUM immediately (no kt-boundary stall),
# optional DVE fast-exp (Schraudolph) to offload the saturated ScalarE.
import sys

sys.path.insert(0, "/opt/trn_rl_repo")
import numpy as np

B, T, C = 2, 2048, 2048
NH, G, HS = 32, 8, 64
QPK = NH // G
NCORES = 8
CHUNK = 256
NCH = T // CHUNK          # 8 chunks per batch
NSLOT_B, NSLOT_A = 16, 8  # uniform kv 128-slots for the two q-chunks
NPREF = 14                # gathered prefix slots resident in SBUF

FAST_EXP = True
# fast-exp slots (always-valid on every core so no per-core validity is
# needed on this path): qcB slots 0..4 (0,1 are the masked local slots),
# qcA slots 0,1 (local).
FAST_SLOTS = {1: (0, 1, 2, 3, 4), 0: (0, 1)}
A16 = 0.125 * 1.4426950408889634 * 128.0   # score -> bf16-exponent scale
B16 = (127.0 - 0.043677) * 128.0           # Schraudolph bias (bf16 grid)

# Head ordering: q-tile t holds (EVEN_HEADS[t] at partitions 0-63,
# ODD_HEADS[t] at 64-127) so the K-slice partition base (g%2)*64 always
# matches the q-slice base.
EVEN_HEADS = [h for h in range(NH) if (h // QPK) % 2 == 0]
ODD_HEADS = [h for h in range(NH) if (h // QPK) % 2 == 1]


def tok_ids(r):
    a = list(range(r * CHUNK, (r + 1) * CHUNK))
    b = list(range((7 - r) * CHUNK, (8 - r) * CHUNK))
    return np.array(a + b, dtype=np.int64)


def perm_q():
    # reordered q feature j = t*128 + s*64 + d  ->  original attn_w row
    p = np.zeros(NH * HS, dtype=np.int64)
    for t in range(16):
        for s, h in ((0, EVEN_HEADS[t]), (1, ODD_HEADS[t])):
            g, qi = h // QPK, h % QPK
            for d in range(HS):
                p[t * 128 + s * 64 + d] = g * 384 + qi * 64 + d
    return p


def perm_k():
    p = np.zeros(G * HS, dtype=np.int64)
    for g in range(G):
        for d in range(HS):
            p[g * 64 + d] = g * 384 + 256 + d
    return p


def perm_v():
    p = np.zeros(G * HS, dtype=np.int64)
    for g in range(G):
        for d in range(HS):
            p[g * 64 + d] = g * 384 + 320 + d
    return p


def perm_y():
    # y^T row i = t*128 + s*64 + d -> proj_w column h*64+d
    p = np.zeros(NH * HS, dtype=np.int64)
    for t in range(16):
        for s, h in ((0, EVEN_HEADS[t]), (1, ODD_HEADS[t])):
            for d in range(HS):
                p[t * 128 + s * 64 + d] = h * 64 + d
    return p


def slot_src(s):
    # gathered prefix slot s (kv 128-chunk index s) -> (rank, 256-chunk pos, col128)
    ci = s // 2
    if ci < 4:
        return ci, 0, s % 2
    return 7 - ci, 1, s % 2


def valid_tables(r):
    # validB[s]: qcB (chunk 7-r) prefix slot s valid; validA[s]: qcA (chunk r)
    vB = np.zeros(16, np.float32)
    vA = np.zeros(16, np.float32)
    for s in range(NPREF):
        vB[s] = 1.0 if s <= 13 - 2 * r else 0.0
    for s in range(6):
        vA[s] = 1.0 if s <= 2 * r - 1 else 0.0
    return vA, vB


def host_masks():
    i = np.arange(128)[:, None]
    j = np.arange(256)[None, :]
    m0 = (i <= j).astype(np.float32)
    m1 = (128 + i <= j).astype(np.float32)
    return m0, m1


_PROG = {}


def _build_program():
    if "nc" in _PROG:
        return _PROG
    import concourse.bass as bass
    import concourse.tile as tile
    from concourse import bacc, mybir
    from contextlib import ExitStack

    f32 = mybir.dt.float32
    i16 = mybir.dt.int16
    fr = mybir.dt.bfloat16
    AF = mybir.ActivationFunctionType
    ALU = mybir.AluOpType

    nc = bacc.Bacc("TRN2", target_bir_lowering=False, debug=False, num_devices=NCORES)

    # strip-major weights: one contiguous [128, 16*128] block per strip
    xT_d = nc.dram_tensor("xT", [128, 16, 512], fr, kind="ExternalInput").ap()
    wqk_d = nc.dram_tensor("wqk", [20, 128, 16, 128], fr, kind="ExternalInput").ap()
    wv_d = nc.dram_tensor("wv", [4, 128, 16, 128], fr, kind="ExternalInput").ap()
    pw_d = nc.dram_tensor("pw", [128, 4, 16, 512], fr, kind="ExternalInput").ap()
    bqk_d = nc.dram_tensor("bqk", [128, 20], f32, kind="ExternalInput").ap()
    bv_d = nc.dram_tensor("bv", [128, 512], f32, kind="ExternalInput").ap()
    pb_d = nc.dram_tensor("pb", [128, C], f32, kind="ExternalInput").ap()
    cos_d = nc.dram_tensor("cosT2", [128, 512], fr, kind="ExternalInput").ap()
    sin_d = nc.dram_tensor("sinT2s", [128, 512], fr, kind="ExternalInput").ap()
    # additive f32 mask-bias tiles for the fast-exp path: B16 (pass),
    # -1e6 (masked); [0]=slot0, [1]=slot1
    mb_d = nc.dram_tensor("mbias", [128, 2, 1024], f32, kind="ExternalInput").ap()
    # additive exp bias per (qc, slot): 0 valid, -30000 invalid
    vb_d = nc.dram_tensor("vbias", [128, 32], f32, kind="ExternalInput").ap()
    vones_d = nc.dram_tensor("vones", [128, 4, 8], fr, kind="ExternalInput").ap()
    out_d = nc.dram_tensor("out", [512, C], f32, kind="ExternalOutput").ap()

    GROUPS = [[0, 1, 2, 3], [4, 5, 6, 7]]

    dbg = {}
    with tile.TileContext(nc) as tc:
        with ExitStack() as ctx:
            consts = ctx.enter_context(tc.tile_pool(name="consts", bufs=1))
            qy = ctx.enter_context(tc.tile_pool(name="qy", bufs=1))
            kvloc = ctx.enter_context(tc.tile_pool(name="kvloc", bufs=1))
            pwp = ctx.enter_context(tc.tile_pool(name="pwp", bufs=2))
            dram = ctx.enter_context(tc.tile_pool(name="dram", bufs=1, space="DRAM"))

            cos_s = consts.tile([128, 512], fr)
            sin_s = consts.tile([128, 512], fr)
            bqk_s = consts.tile([128, 20], f32)
            bv_s = consts.tile([128, 512], f32)
            pb_s = consts.tile([128, C], f32)
            mb_s = consts.tile([128, 2, 1024], f32)
            vb_s = consts.tile([128, 32], f32)

            qT_s = qy.tile([128, 16, 512], fr)
            yT_s = qy.tile([128, 16, 512], fr)
            kTl_s = kvloc.tile([128, 4, 512], fr)
            vAl_s = kvloc.tile([128, 4, 8, 65], fr)
            kg_s = kvloc.tile([128, NPREF, 4, 128], fr)
            vg_s = kvloc.tile([128, NPREF, 8, 65], fr)


            dbg.update(qT=qT_s, yT=yT_s, kTl=kTl_s, vAl=vAl_s, kg=kg_s, vg=vg_s)
            k_loc = dram.tile([4, 128, 512], fr)
            v_loc = dram.tile([4, 128, 8, 65], fr)
            k_gat = dram.tile([4, 4, 128, 512], fr)
            v_gat = dram.tile([4, 4, 128, 8, 65], fr)

            # ---------------- phase 0: QKV projections ----------------
            with ExitStack() as p0:
                xp = p0.enter_context(tc.tile_pool(name="xp", bufs=1))
                wqp = p0.enter_context(tc.tile_pool(name="wqp", bufs=6))
                rp = p0.enter_context(tc.tile_pool(name="rp", bufs=2))
                ps0 = p0.enter_context(tc.tile_pool(name="ps0", bufs=2, space="PSUM"))

                # x first (needed by every matmul), then consts on other queues
                xT_s = xp.tile([128, 16, 512], fr)
                nc.scalar.dma_start(out=xT_s, in_=xT_d)
                nc.gpsimd.dma_start(out=cos_s, in_=cos_d)
                nc.gpsimd.dma_start(out=sin_s, in_=sin_d)
                nc.gpsimd.dma_start(out=bqk_s, in_=bqk_d)
                nc.gpsimd.dma_start(out=bv_s, in_=bv_d)
                nc.gpsimd.dma_start(out=pb_s, in_=pb_d)
                nc.gpsimd.dma_start(out=mb_s, in_=mb_d)
                nc.gpsimd.dma_start(out=vb_s, in_=vb_d)
                wv_all = xp.tile([128, 4, 16, 128], fr)
                for s4 in range(4):
                    nc.scalar.dma_start(out=wv_all[:, s4], in_=wv_d[s4])

                def rope_into(dst, ps, bias_col):
                    # dst[128, 512] <- rope(ps + bias); the 32-row
                    # rotate-half swap via SBUF->SBUF DMAs.
                    tb = rp.tile([128, 512], fr, tag="tb", name="tb")
                    nc.scalar.activation(tb, ps, AF.Identity, bias=bias_col)
                    t2 = rp.tile([128, 512], fr, tag="t2", name="t2")
                    nc.vector.tensor_mul(t2, tb, sin_s)
                    tcs = rp.tile([128, 512], fr, tag="tc", name="tcs")
                    nc.vector.tensor_mul(tcs, tb, cos_s)
                    t2s = rp.tile([128, 512], fr, tag="t2s", name="t2s")
                    for b0 in (0, 32, 64, 96):
                        d0 = b0 ^ 32
                        nc.vector.tensor_copy(t2s[d0:d0 + 32, :], t2[b0:b0 + 32, :])
                    nc.vector.tensor_add(dst, t2s, tcs)

                def qk_strip(dst, strip, bias_col):
                    ws = wqp.tile([128, 16, 128], fr, tag="wq", name="ws")
                    nc.sync.dma_start(out=ws, in_=wqk_d[strip])
                    ps = ps0.tile([128, 512], f32, tag="pk", bufs=4, name="ps")
                    for kc in range(16):
                        nc.tensor.matmul(ps, ws[:, kc, :], xT_s[:, kc, :],
                                         start=(kc == 0), stop=(kc == 15))
                    rope_into(dst, ps, bias_col)

                # K^T tiles (strips 0-3 of wqk_d = K columns, host-reordered)
                for kt in range(4):
                    qk_strip(kTl_s[:, kt, :], kt, bqk_s[:, 16 + kt:17 + kt])
                nc.sync.dma_start(out=k_loc.rearrange("k p t -> p k t"), in_=kTl_s)
                nc.gpsimd.collective_compute(
                    "AllGather", mybir.AluOpType.bypass, replica_groups=GROUPS,
                    ins=[k_loc.opt()], outs=[k_gat.opt()])

                # V tiles (natural layout, bias, ones column)
                psv = [ps0.tile([128, 512], f32, tag="pv", bufs=4, name=f"psv{mt}") for mt in range(4)]
                for kc in range(16):
                    for mt in range(4):
                        nc.tensor.matmul(psv[mt], xT_s[:, kc, mt * 128:(mt + 1) * 128],
                                         wv_all[:, :, kc, :],
                                         start=(kc == 0), stop=(kc == 15))
                nc.scalar.dma_start(out=vAl_s[:, :, :, 64:65],
                                    in_=vones_d.rearrange("p c (g o) -> p c g o", o=1))
                for mt in range(4):
                    nc.vector.tensor_add(
                        vAl_s[:, mt, :, 0:64],
                        psv[mt].rearrange("p (g d) -> p g d", g=8),
                        bv_s.rearrange("p (g d) -> p g d", g=8))
                nc.sync.dma_start(out=v_loc.rearrange("c p g d -> p c g d"), in_=vAl_s)
                nc.gpsimd.collective_compute(
                    "AllGather", mybir.AluOpType.bypass, replica_groups=GROUPS,
                    ins=[v_loc.opt()], outs=[v_gat.opt()])

                # Q^T tiles (strips 4-19)
                for qt in range(16):
                    qk_strip(qT_s[:, qt, :], 4 + qt, bqk_s[:, qt:qt + 1])


            # ---------------- phase 1: attention ----------------
            with ExitStack() as pa:
                ep = pa.enter_context(tc.tile_pool(name="ep", bufs=6))
                nrm = pa.enter_context(tc.tile_pool(name="nrm", bufs=3))
                psA = pa.enter_context(tc.tile_pool(name="psA", bufs=1, space="PSUM"))

                for s in range(NPREF):
                    rk, cp, half = slot_src(s)
                    eng = (nc.scalar, nc.sync, nc.gpsimd)[s % 3]
                    eng.dma_start(out=kg_s[:, s, :, :],
                                  in_=k_gat[rk, :, :, cp * 256 + half * 128: cp * 256 + (half + 1) * 128]
                                  .rearrange("k p t -> p k t"))
                    eng.dma_start(out=vg_s[:, s, :, :], in_=v_gat[rk, cp * 2 + half, :, :, :])

                def attend(qc, nslots):
                    fast = FAST_SLOTS[qc] if FAST_EXP else ()
                    sc = [s for s in range(nslots) if s not in fast]
                    fa = list(fast)
                    # interleave: ~2 scalar-exp slots between fast slots
                    order = []
                    while sc or fa:
                        order.extend(sc[:2])
                        sc = sc[2:]
                        if fa:
                            order.append(fa.pop(0))
                    for kt in range(4):
                        avs = [psA.tile([65, 1024], f32, tag=f"av{gi}", bufs=1,
                                        name=f"ps_av{gi}") for gi in range(2)]
                        for si, slot in enumerate(order):
                            Ks, Vs = [], []
                            for gi in range(2):
                                g = 2 * kt + gi
                                if slot < 2:
                                    Ks.append(kTl_s[gi * 64:(gi + 1) * 64, kt, qc * 256 + slot * 128: qc * 256 + (slot + 1) * 128])
                                    Vs.append(vAl_s[:, qc * 2 + slot, g, :])
                                else:
                                    Ks.append(kg_s[gi * 64:(gi + 1) * 64, slot - 2, kt, :])
                                    Vs.append(vg_s[:, slot - 2, g, :])
                            pss = [psA.tile([128, 1024], f32, tag="s", bufs=2, name=f"ps_s{gi}")
                                   for gi in range(2)]
                            for pair in range(2):
                                tA = kt * 4 + 2 * pair
                                for gi in range(2):
                                    nc.tensor.matmul(pss[gi][:, pair * 512:(pair + 1) * 512], Ks[gi],
                                                     qT_s[gi * 64:(gi + 1) * 64, tA:tA + 2, qc * 256:(qc + 1) * 256],
                                                     start=True, stop=True,
                                                     tile_position=(gi * 64, 0))
                            es = []
                            vcol = vb_s[:, (1 - qc) * 16 + slot:(1 - qc) * 16 + slot + 1]
                            for gi in range(2):
                                if slot in fast:
                                    # Schraudolph exp2 on DVE: bf16 bit
                                    # pattern via int16 round of affine map
                                    tf = ep.tile([128, 1024], f32, tag="tf", bufs=3, name="tf")
                                    if slot < 2:
                                        nc.vector.scalar_tensor_tensor(
                                            tf, pss[gi], A16, mb_s[:, slot, :],
                                            ALU.mult, ALU.add)
                                    else:
                                        nc.vector.tensor_scalar(
                                            tf, pss[gi], A16, B16, ALU.mult, ALU.add)
                                    e16 = ep.tile([128, 1024], i16, tag="e", name="e16")
                                    nc.gpsimd.tensor_scalar(
                                        e16, tf, 0.0, None, ALU.max)
                                    es.append(e16.bitcast(fr))
                                else:
                                    e = ep.tile([128, 1024], fr, tag="e", name="e")
                                    nc.scalar.activation(e, pss[gi], AF.Exp,
                                                         scale=0.125, bias=vcol)
                                    es.append(e)
                            for gi in range(2):
                                for pair in range(2):
                                    nc.tensor.matmul(avs[gi][:, pair * 512:(pair + 1) * 512],
                                                     Vs[gi], es[gi][:, pair * 512:(pair + 1) * 512],
                                                     start=(si == 0), stop=(si == nslots - 1))
                        # evacuate PSUM promptly, normalize from SBUF
                        # (reciprocal must run at partition 0: cross-
                        # partition DVE reciprocal reads garbage on HW)
                        for gi in range(2):
                            ya = nrm.tile([64, 1024], f32, tag="ya", bufs=2, name="ya")
                            nc.scalar.copy(ya, avs[gi][0:64, :])
                            rd = nrm.tile([1, 1024], f32, tag="rd", bufs=2, name="rd")
                            nc.vector.tensor_copy(rd, avs[gi][64:65, :])
                            r_ = nrm.tile([1, 1024], f32, tag="r", bufs=2, name="r_")
                            nc.vector.reciprocal_approx_fast(r_, rd)
                            rb = nrm.tile([64, 1024], f32, tag="rb", bufs=2, name="rb")
                            nc.gpsimd.partition_broadcast(rb, r_)
                            tA = kt * 4
                            nc.vector.tensor_mul(
                                yT_s[gi * 64:(gi + 1) * 64, tA:tA + 4, qc * 256:(qc + 1) * 256],
                                ya.rearrange("p (j t) -> p j t", j=4),
                                rb.rearrange("p (j t) -> p j t", j=4))

                attend(1, NSLOT_B)
                attend(0, NSLOT_A)

            # ---------------- phase 2: output projection ----------------
            with ExitStack() as pp:
                pr = pp.enter_context(tc.tile_pool(name="pr", bufs=4))
                psP = pp.enter_context(tc.tile_pool(name="psP", bufs=8, space="PSUM"))
                for n in range(4):
                    pwn = pwp.tile([128, 16, 512], fr, tag="pwn", name=f"pwn{n}")
                    nc.sync.dma_start(out=pwn, in_=pw_d[:, n])
                    pss = [psP.tile([128, 512], f32, tag="pp", name=f"pss{mt}") for mt in range(4)]
                    for kd in range(16):
                        for mt in range(4):
                            nc.tensor.matmul(pss[mt], yT_s[:, kd, mt * 128:(mt + 1) * 128],
                                             pwn[:, kd, :],
                                             start=(kd == 0), stop=(kd == 15))
                    for mt in range(4):
                        ost = pr.tile([128, 512], f32, tag="ost", name="ost")
                        nc.vector.tensor_add(ost, pss[mt], pb_s[:, n * 512:(n + 1) * 512])
                        eng = (nc.sync, nc.scalar, nc.gpsimd, nc.sync)[mt]
                        eng.dma_start(out=out_d[mt * 128:(mt + 1) * 128, n * 512:(n + 1) * 512], in_=ost)

    nc.compile()
    _PROG["nc"] = nc
    _PROG["dbg"] = dbg
    return _PROG


def make_in_maps(x, cos, sin, attn_w, attn_b, proj_w, proj_b):
    import ml_dtypes
    mmt = ml_dtypes.bfloat16

    x = np.asarray(x, np.float32)
    cos = np.asarray(cos, np.float32)
    sin = np.asarray(sin, np.float32)
    attn_w = np.asarray(attn_w, np.float32)
    attn_b = np.asarray(attn_b, np.float32)
    proj_w = np.asarray(proj_w, np.float32)
    proj_b = np.asarray(proj_b, np.float32)

    pq, pk, pv, py = perm_q(), perm_k(), perm_v(), perm_y()
    pqk = np.concatenate([pq, pk])
    wqkT = np.ascontiguousarray(attn_w[pqk, :].T)          # [2048, 2560]
    wvT = np.ascontiguousarray(attn_w[pv, :].T)            # [2048, 512]
    pwT = np.ascontiguousarray(proj_w.T[py, :])            # [2048, 2048]

    # strip-major wqk: strip order = [K strips 16-19, Q strips 0-15];
    # wqk[s, p, j, q] = wqkT[j*128+p, col0(s)+q]
    w5 = wqkT.reshape(16, 128, 20, 128)                    # [j, p, strip, q]
    w5 = w5.transpose(2, 1, 0, 3)                          # [strip, p, j, q]
    order = [16, 17, 18, 19] + list(range(16))
    wqk = np.ascontiguousarray(w5[order]).astype(mmt)      # [20, 128, 16, 128]
    wv5 = wvT.reshape(16, 128, 4, 128).transpose(2, 1, 0, 3)
    wv = np.ascontiguousarray(wv5).astype(mmt)             # [4, 128, 16, 128]
    # pw[p, n, kd, q] = pwT[kd*128+p, n*512+q]
    pw6 = pwT.reshape(16, 128, 4, 512).transpose(1, 2, 0, 3)
    pw = np.ascontiguousarray(pw6).astype(mmt)             # [128, 4, 16, 512]

    bqk = np.ascontiguousarray(attn_b[pqk].reshape(20, 128).T)   # [128, 20]
    bv = np.tile(attn_b[pv][None, :], (128, 1)).astype(np.float32)
    pb = np.tile(proj_b[None, :], (128, 1)).astype(np.float32)
    m0, m1 = host_masks()
    # fast-exp additive bias tiles
    mbias = np.empty((128, 2, 1024), np.float32)
    mbias[:, 0, :] = np.where(np.concatenate([m0] * 4, axis=1) > 0, B16, -1e6)
    mbias[:, 1, :] = np.where(np.concatenate([m1] * 4, axis=1) > 0, B16, -1e6)

    in_maps = []
    for c in range(NCORES):
        b, r = c // 4, c % 4
        ids = tok_ids(r)
        xT = x[b, ids, :].T.reshape(16, 128, 512).transpose(1, 0, 2)
        xT = np.ascontiguousarray(xT).astype(mmt)          # [128, 16, 512]
        cl = cos[ids, :].T                                 # [64, 512]
        sl = sin[ids, :].T.copy()
        sl[32:] *= -1.0
        cosT2 = np.concatenate([cl, cl], axis=0).astype(mmt)
        sinT2s = np.concatenate([sl, sl], axis=0).astype(mmt)
        vA, vB = valid_tables(r)
        # vbias is indexed by the attend() LOOP slot: slots 0,1 are the
        # always-valid local slots; slot s>=2 maps to prefix slot s-2.
        vbias = np.full((128, 32), -30000.0, np.float32)
        vbias[:, 0:2] = 0.0
        vbias[:, 16:18] = 0.0
        for s in range(2, 16):
            if vB[s - 2] > 0:
                vbias[:, s] = 0.0
        for s in range(2, 8):
            if vA[s - 2] > 0:
                vbias[:, 16 + s] = 0.0
        in_maps.append({
            "xT": xT, "wqk": wqk, "wv": wv, "pw": pw,
            "bqk": bqk, "bv": bv, "pb": pb,
            "cosT2": cosT2, "sinT2s": sinT2s,
            "mbias": mbias, "vbias": vbias,
            "vones": np.ones((128, 4, 8), mmt),
        })
    return in_maps


def assemble_output(results):
    out = np.zeros((B, T, C), np.float32)
    for c in range(NCORES):
        b, r = c // 4, c % 4
        ids = tok_ids(r)
        out[b, ids, :] = results[c]["out"]
    return out


def kernel(**inputs):
    from concourse.bass_utils import run_bass_kernel_spmd

    prog = _build_program()
    in_maps = make_in_maps(**inputs)
    res = run_bass_kernel_spmd(prog["nc"], in_maps, list(range(NCORES)))
    return assemble_output(res.results)


if __name__ == "__main__":
    import reference

    inputs = {k: np.asarray(v) for k, v in reference.setup_inputs().items()}
    expected = np.asarray(reference.reference(**inputs))
    actual = kernel(**inputs)
    err = np.abs(actual - expected).max()
    rel = np.abs(actual - expected).max() / np.abs(expected).max()
    print(f"abs={err:.3e} rel={rel:.3e}")


# revision 24
# speedup vs baseline: 2.3718x; 2.3718x over previous
# Causal self-attention (GQA, RoPE) on 8 NeuronCores.
#
# Sharding: sequence-parallel. Core c = (batch b = c//4, role r = c%4).
# Each batch's 2048 tokens are split into 8 chunks of 256; role r owns
# chunks {r, 7-r} (zigzag, balances causal work). Each core computes
# QKV for its 512 tokens, AllGathers RoPE'd K^T and ones-augmented V
# within its 4-core batch group, runs causal attention for a uniform
# 24-slot schedule (SPMD needs identical instruction streams; per-core
# causal validity is data: invalid slots get an additive -30000 exp
# bias so their weights are exactly zero), then projects its own
# token rows. No reduction needed after proj.
#
# v2: weight streaming in contiguous 128-col strips (PE never waits on
# HBM), proj weights prefetched to SBUF during attention, softmax
# denominators evacuated from PSUM immediately (no kt-boundary stall),
# optional DVE fast-exp (Schraudolph) to offload the saturated ScalarE.
import sys

sys.path.insert(0, "/opt/trn_rl_repo")
import numpy as np

B, T, C = 2, 2048, 2048
NH, G, HS = 32, 8, 64
QPK = NH // G
NCORES = 8
CHUNK = 256
NCH = T // CHUNK          # 8 chunks per batch
NSLOT_B, NSLOT_A = 16, 8  # uniform kv 128-slots for the two q-chunks
NPREF = 14                # gathered prefix slots resident in SBUF

FAST_EXP = True
# fast-exp slots (always-valid on every core so no per-core validity is
# needed on this path): qcB slots 0..4 (0,1 are the masked local slots),
# qcA slots 0,1 (local).
FAST_SLOTS = {1: (0, 1, 2, 3, 4), 0: (0, 1)}
A16 = 0.125 * 1.4426950408889634 * 128.0   # score -> bf16-exponent scale
B16 = (127.0 - 0.043677) * 128.0           # Schraudolph bias (bf16 grid)

# Head ordering: q-tile t holds (EVEN_HEADS[t] at partitions 0-63,
# ODD_HEADS[t] at 64-127) so the K-slice partition base (g%2)*64 always
# matches the q-slice base.
EVEN_HEADS = [h for h in range(NH) if (h // QPK) % 2 == 0]
ODD_HEADS = [h for h in range(NH) if (h // QPK) % 2 == 1]


def tok_ids(r):
    a = list(range(r * CHUNK, (r + 1) * CHUNK))
    b = list(range((7 - r) * CHUNK, (8 - r) * CHUNK))
    return np.array(a + b, dtype=np.int64)


def perm_q():
    # reordered q feature j = t*128 + s*64 + d  ->  original attn_w row
    p = np.zeros(NH * HS, dtype=np.int64)
    for t in range(16):
        for s, h in ((0, EVEN_HEADS[t]), (1, ODD_HEADS[t])):
            g, qi = h // QPK, h % QPK
            for d in range(HS):
                p[t * 128 + s * 64 + d] = g * 384 + qi * 64 + d
    return p


def perm_k():
    p = np.zeros(G * HS, dtype=np.int64)
    for g in range(G):
        for d in range(HS):
            p[g * 64 + d] = g * 384 + 256 + d
    return p


def perm_v():
    p = np.zeros(G * HS, dtype=np.int64)
    for g in range(G):
        for d in range(HS):
            p[g * 64 + d] = g * 384 + 320 + d
    return p


def perm_y():
    # y^T row i = t*128 + s*64 + d -> proj_w column h*64+d
    p = np.zeros(NH * HS, dtype=np.int64)
    for t in range(16):
        for s, h in ((0, EVEN_HEADS[t]), (1, ODD_HEADS[t])):
            for d in range(HS):
                p[t * 128 + s * 64 + d] = h * 64 + d
    return p


def slot_src(s):
    # gathered prefix slot s (kv 128-chunk index s) -> (rank, 256-chunk pos, col128)
    ci = s // 2
    if ci < 4:
        return ci, 0, s % 2
    return 7 - ci, 1, s % 2


def valid_tables(r):
    # validB[s]: qcB (chunk 7-r) prefix slot s valid; validA[s]: qcA (chunk r)
    vB = np.zeros(16, np.float32)
    vA = np.zeros(16, np.float32)
    for s in range(NPREF):
        vB[s] = 1.0 if s <= 13 - 2 * r else 0.0
    for s in range(6):
        vA[s] = 1.0 if s <= 2 * r - 1 else 0.0
    return vA, vB


def host_masks():
    i = np.arange(128)[:, None]
    j = np.arange(256)[None, :]
    m0 = (i <= j).astype(np.float32)
    m1 = (128 + i <= j).astype(np.float32)
    return m0, m1


_PROG = {}


def _build_program():
    if "nc" in _PROG:
        return _PROG
    import concourse.bass as bass
    import concourse.tile as tile
    from concourse import bacc, mybir
    from contextlib import ExitStack

    f32 = mybir.dt.float32
    i16 = mybir.dt.int16
    fr = mybir.dt.bfloat16
    AF = mybir.ActivationFunctionType
    ALU = mybir.AluOpType

    nc = bacc.Bacc("TRN2", target_bir_lowering=False, debug=False, num_devices=NCORES)

    # strip-major weights: one contiguous [128, 16*128] block per strip
    xT_d = nc.dram_tensor("xT", [128, 16, 512], fr, kind="ExternalInput").ap()
    wqk_d = nc.dram_tensor("wqk", [20, 128, 16, 128], fr, kind="ExternalInput").ap()
    wv_d = nc.dram_tensor("wv", [4, 128, 16, 128], fr, kind="ExternalInput").ap()
    pw_d = nc.dram_tensor("pw", [128, 4, 16, 512], fr, kind="ExternalInput").ap()
    bqk_d = nc.dram_tensor("bqk", [128, 20], f32, kind="ExternalInput").ap()
    bv_d = nc.dram_tensor("bv", [128, 512], f32, kind="ExternalInput").ap()
    pb_d = nc.dram_tensor("pb", [128, C], f32, kind="ExternalInput").ap()
    cos_d = nc.dram_tensor("cosT2", [128, 512], fr, kind="ExternalInput").ap()
    sin_d = nc.dram_tensor("sinT2s", [128, 512], fr, kind="ExternalInput").ap()
    # additive f32 mask-bias tiles for the fast-exp path: B16 (pass),
    # -1e6 (masked); [0]=slot0, [1]=slot1
    mb_d = nc.dram_tensor("mbias", [128, 2, 1024], f32, kind="ExternalInput").ap()
    # additive exp bias per (qc, slot): 0 valid, -30000 invalid
    vb_d = nc.dram_tensor("vbias", [128, 32], f32, kind="ExternalInput").ap()
    vones_d = nc.dram_tensor("vones", [128, 4, 8], fr, kind="ExternalInput").ap()
    out_d = nc.dram_tensor("out", [512, C], f32, kind="ExternalOutput").ap()

    GROUPS = [[0, 1, 2, 3], [4, 5, 6, 7]]

    dbg = {}
    with tile.TileContext(nc) as tc:
        with ExitStack() as ctx:
            consts = ctx.enter_context(tc.tile_pool(name="consts", bufs=1))
            qy = ctx.enter_context(tc.tile_pool(name="qy", bufs=1))
            kvloc = ctx.enter_context(tc.tile_pool(name="kvloc", bufs=1))
            pwp = ctx.enter_context(tc.tile_pool(name="pwp", bufs=2))
            dram = ctx.enter_context(tc.tile_pool(name="dram", bufs=1, space="DRAM"))

            cos_s = consts.tile([128, 512], fr)
            sin_s = consts.tile([128, 512], fr)
            bqk_s = consts.tile([128, 20], f32)
            bv_s = consts.tile([128, 512], f32)
            pb_s = consts.tile([128, C], f32)
            mb_s = consts.tile([128, 2, 1024], f32)
            vb_s = consts.tile([128, 32], f32)

            qT_s = qy.tile([128, 16, 512], fr)
            yT_s = qy.tile([128, 16, 512], fr)
            kTl_s = kvloc.tile([128, 4, 512], fr)
            vAl_s = kvloc.tile([128, 4, 8, 65], fr)
            kg_s = kvloc.tile([128, NPREF, 4, 128], fr)
            vg_s = kvloc.tile([128, NPREF, 8, 65], fr)


            dbg.update(qT=qT_s, yT=yT_s, kTl=kTl_s, vAl=vAl_s, kg=kg_s, vg=vg_s)
            k_loc = dram.tile([4, 128, 512], fr)
            v_loc = dram.tile([4, 128, 8, 65], fr)
            k_gat = dram.tile([4, 4, 128, 512], fr)
            v_gat = dram.tile([4, 4, 128, 8, 65], fr)

            # ---------------- phase 0: QKV projections ----------------
            with ExitStack() as p0:
                xp = p0.enter_context(tc.tile_pool(name="xp", bufs=1))
                wqp = p0.enter_context(tc.tile_pool(name="wqp", bufs=6))
                rp = p0.enter_context(tc.tile_pool(name="rp", bufs=2))
                ps0 = p0.enter_context(tc.tile_pool(name="ps0", bufs=2, space="PSUM"))

                # x first (needed by every matmul), then consts on other queues
                xT_s = xp.tile([128, 16, 512], fr)
                nc.scalar.dma_start(out=xT_s, in_=xT_d)
                nc.gpsimd.dma_start(out=cos_s, in_=cos_d)
                nc.gpsimd.dma_start(out=sin_s, in_=sin_d)
                nc.gpsimd.dma_start(out=bqk_s, in_=bqk_d)
                nc.gpsimd.dma_start(out=bv_s, in_=bv_d)
                nc.gpsimd.dma_start(out=pb_s, in_=pb_d)
                nc.gpsimd.dma_start(out=mb_s, in_=mb_d)
                nc.gpsimd.dma_start(out=vb_s, in_=vb_d)
                wv_all = xp.tile([128, 4, 16, 128], fr)
                for s4 in range(4):
                    nc.scalar.dma_start(out=wv_all[:, s4], in_=wv_d[s4])

                def rope_into(dst, ps, bias_col):
                    # dst[128, 512] <- rope(ps + bias); the 32-row
                    # rotate-half swap via SBUF->SBUF DMAs.
                    tb = rp.tile([128, 512], fr, tag="tb", name="tb")
                    nc.scalar.activation(tb, ps, AF.Identity, bias=bias_col)
                    t2 = rp.tile([128, 512], fr, tag="t2", name="t2")
                    nc.vector.tensor_mul(t2, tb, sin_s)
                    tcs = rp.tile([128, 512], fr, tag="tc", name="tcs")
                    nc.vector.tensor_mul(tcs, tb, cos_s)
                    t2s = rp.tile([128, 512], fr, tag="t2s", name="t2s")
                    for eng, b0 in ((nc.sync, 0), (nc.scalar, 64)):
                        eng.dma_start(out=t2s[b0:b0 + 32, :], in_=t2[b0 + 32:b0 + 64, :])
                        eng.dma_start(out=t2s[b0 + 32:b0 + 64, :], in_=t2[b0:b0 + 32, :])
                    nc.vector.tensor_add(dst, t2s, tcs)

                def qk_strip(dst, strip, bias_col):
                    ws = wqp.tile([128, 16, 128], fr, tag="wq", name="ws")
                    nc.sync.dma_start(out=ws, in_=wqk_d[strip])
                    ps = ps0.tile([128, 512], f32, tag="pk", bufs=4, name="ps")
                    for kc in range(16):
                        nc.tensor.matmul(ps, ws[:, kc, :], xT_s[:, kc, :],
                                         start=(kc == 0), stop=(kc == 15))
                    rope_into(dst, ps, bias_col)

                # K^T tiles (strips 0-3 of wqk_d = K columns, host-reordered)
                for kt in range(4):
                    qk_strip(kTl_s[:, kt, :], kt, bqk_s[:, 16 + kt:17 + kt])
                nc.sync.dma_start(out=k_loc.rearrange("k p t -> p k t"), in_=kTl_s)
                nc.gpsimd.collective_compute(
                    "AllGather", mybir.AluOpType.bypass, replica_groups=GROUPS,
                    ins=[k_loc.opt()], outs=[k_gat.opt()])

                # V tiles (natural layout, bias, ones column)
                psv = [ps0.tile([128, 512], f32, tag="pv", bufs=4, name=f"psv{mt}") for mt in range(4)]
                for kc in range(16):
                    for mt in range(4):
                        nc.tensor.matmul(psv[mt], xT_s[:, kc, mt * 128:(mt + 1) * 128],
                                         wv_all[:, :, kc, :],
                                         start=(kc == 0), stop=(kc == 15))
                nc.scalar.dma_start(out=vAl_s[:, :, :, 64:65],
                                    in_=vones_d.rearrange("p c (g o) -> p c g o", o=1))
                for mt in range(4):
                    nc.vector.tensor_add(
                        vAl_s[:, mt, :, 0:64],
                        psv[mt].rearrange("p (g d) -> p g d", g=8),
                        bv_s.rearrange("p (g d) -> p g d", g=8))
                nc.sync.dma_start(out=v_loc.rearrange("c p g d -> p c g d"), in_=vAl_s)
                nc.gpsimd.collective_compute(
                    "AllGather", mybir.AluOpType.bypass, replica_groups=GROUPS,
                    ins=[v_loc.opt()], outs=[v_gat.opt()])

                # Q^T tiles (strips 4-19)
                for qt in range(16):
                    qk_strip(qT_s[:, qt, :], 4 + qt, bqk_s[:, qt:qt + 1])


            # ---------------- phase 1: attention ----------------
            with ExitStack() as pa:
                ep = pa.enter_context(tc.tile_pool(name="ep", bufs=6))
                nrm = pa.enter_context(tc.tile_pool(name="nrm", bufs=3))
                psA = pa.enter_context(tc.tile_pool(name="psA", bufs=1, space="PSUM"))

                for s in range(NPREF):
                    rk, cp, half = slot_src(s)
                    eng = (nc.scalar, nc.sync, nc.gpsimd)[s % 3]
                    eng.dma_start(out=kg_s[:, s, :, :],
                                  in_=k_gat[rk, :, :, cp * 256 + half * 128: cp * 256 + (half + 1) * 128]
                                  .rearrange("k p t -> p k t"))
                    eng.dma_start(out=vg_s[:, s, :, :], in_=v_gat[rk, cp * 2 + half, :, :, :])

                def attend(qc, nslots):
                    fast = FAST_SLOTS[qc] if FAST_EXP else ()
                    sc = [s for s in range(nslots) if s not in fast]
                    fa = list(fast)
                    # interleave: ~2 scalar-exp slots between fast slots
                    order = []
                    while sc or fa:
                        order.extend(sc[:2])
                        sc = sc[2:]
                        if fa:
                            order.append(fa.pop(0))
                    for kt in range(4):
                        avs = [psA.tile([65, 1024], f32, tag=f"av{gi}", bufs=1,
                                        name=f"ps_av{gi}") for gi in range(2)]
                        for si, slot in enumerate(order):
                            Ks, Vs = [], []
                            for gi in range(2):
                                g = 2 * kt + gi
                                if slot < 2:
                                    Ks.append(kTl_s[gi * 64:(gi + 1) * 64, kt, qc * 256 + slot * 128: qc * 256 + (slot + 1) * 128])
                                    Vs.append(vAl_s[:, qc * 2 + slot, g, :])
                                else:
                                    Ks.append(kg_s[gi * 64:(gi + 1) * 64, slot - 2, kt, :])
                                    Vs.append(vg_s[:, slot - 2, g, :])
                            pss = [psA.tile([128, 1024], f32, tag="s", bufs=2, name=f"ps_s{gi}")
                                   for gi in range(2)]
                            for pair in range(2):
                                tA = kt * 4 + 2 * pair
                                for gi in range(2):
                                    nc.tensor.matmul(pss[gi][:, pair * 512:(pair + 1) * 512], Ks[gi],
                                                     qT_s[gi * 64:(gi + 1) * 64, tA:tA + 2, qc * 256:(qc + 1) * 256],
                                                     start=True, stop=True,
                                                     tile_position=(gi * 64, 0))
                            es = []
                            vcol = vb_s[:, (1 - qc) * 16 + slot:(1 - qc) * 16 + slot + 1]
                            for gi in range(2):
                                if slot in fast:
                                    # Schraudolph exp2 on DVE: bf16 bit
                                    # pattern via int16 round of affine map
                                    tf = ep.tile([128, 1024], f32, tag="tf", bufs=3, name="tf")
                                    if slot < 2:
                                        nc.vector.scalar_tensor_tensor(
                                            tf, pss[gi], A16, mb_s[:, slot, :],
                                            ALU.mult, ALU.add)
                                    else:
                                        nc.vector.tensor_scalar(
                                            tf, pss[gi], A16, B16, ALU.mult, ALU.add)
                                    e16 = ep.tile([128, 1024], i16, tag="e", name="e16")
                                    nc.vector.tensor_scalar(
                                        e16, tf, 0.0, None, ALU.max)
                                    es.append(e16.bitcast(fr))
                                else:
                                    e = ep.tile([128, 1024], fr, tag="e", name="e")
                                    nc.scalar.activation(e, pss[gi], AF.Exp,
                                                         scale=0.125, bias=vcol)
                                    es.append(e)
                            for gi in range(2):
                                for pair in range(2):
                                    nc.tensor.matmul(avs[gi][:, pair * 512:(pair + 1) * 512],
                                                     Vs[gi], es[gi][:, pair * 512:(pair + 1) * 512],
                                                     start=(si == 0), stop=(si == nslots - 1))
                        # evacuate PSUM promptly, normalize from SBUF
                        # (reciprocal must run at partition 0: cross-
                        # partition DVE reciprocal reads garbage on HW)
                        for gi in range(2):
                            ya = nrm.tile([64, 1024], f32, tag="ya", bufs=2, name="ya")
                            nc.scalar.copy(ya, avs[gi][0:64, :])
                            rd = nrm.tile([1, 1024], f32, tag="rd", bufs=2, name="rd")
                            nc.vector.tensor_copy(rd, avs[gi][64:65, :])
                            r_ = nrm.tile([1, 1024], f32, tag="r", bufs=2, name="r_")
                            nc.vector.reciprocal_approx_fast(r_, rd)
                            rb = nrm.tile([64, 1024], f32, tag="rb", bufs=2, name="rb")
                            nc.gpsimd.partition_broadcast(rb, r_)
                            tA = kt * 4
                            nc.vector.tensor_mul(
                                yT_s[gi * 64:(gi + 1) * 64, tA:tA + 4, qc * 256:(qc + 1) * 256],
                                ya.rearrange("p (j t) -> p j t", j=4),
                                rb.rearrange("p (j t) -> p j t", j=4))

                attend(1, NSLOT_B)
                attend(0, NSLOT_A)

            # ---------------- phase 2: output projection ----------------
            with ExitStack() as pp:
                pr = pp.enter_context(tc.tile_pool(name="pr", bufs=4))
                psP = pp.enter_context(tc.tile_pool(name="psP", bufs=8, space="PSUM"))
                for n in range(4):
                    pwn = pwp.tile([128, 16, 512], fr, tag="pwn", name=f"pwn{n}")
                    nc.sync.dma_start(out=pwn, in_=pw_d[:, n])
                    pss = [psP.tile([128, 512], f32, tag="pp", name=f"pss{mt}") for mt in range(4)]
                    for kd in range(16):
                        for mt in range(4):
                            nc.tensor.matmul(pss[mt], yT_s[:, kd, mt * 128:(mt + 1) * 128],
                                             pwn[:, kd, :],
                                             start=(kd == 0), stop=(kd == 15))
                    for mt in range(4):
                        ost = pr.tile([128, 512], f32, tag="ost", name="ost")
                        nc.vector.tensor_add(ost, pss[mt], pb_s[:, n * 512:(n + 1) * 512])
                        eng = (nc.sync, nc.scalar, nc.gpsimd, nc.sync)[mt]
                        eng.dma_start(out=out_d[mt * 128:(mt + 1) * 128, n * 512:(n + 1) * 512], in_=ost)

    nc.compile()
    _PROG["nc"] = nc
    _PROG["dbg"] = dbg
    return _PROG


def make_in_maps(x, cos, sin, attn_w, attn_b, proj_w, proj_b):
    import ml_dtypes
    mmt = ml_dtypes.bfloat16

    x = np.asarray(x, np.float32)
    cos = np.asarray(cos, np.float32)
    sin = np.asarray(sin, np.float32)
    attn_w = np.asarray(attn_w, np.float32)
    attn_b = np.asarray(attn_b, np.float32)
    proj_w = np.asarray(proj_w, np.float32)
    proj_b = np.asarray(proj_b, np.float32)

    pq, pk, pv, py = perm_q(), perm_k(), perm_v(), perm_y()
    pqk = np.concatenate([pq, pk])
    wqkT = np.ascontiguousarray(attn_w[pqk, :].T)          # [2048, 2560]
    wvT = np.ascontiguousarray(attn_w[pv, :].T)            # [2048, 512]
    pwT = np.ascontiguousarray(proj_w.T[py, :])            # [2048, 2048]

    # strip-major wqk: strip order = [K strips 16-19, Q strips 0-15];
    # wqk[s, p, j, q] = wqkT[j*128+p, col0(s)+q]
    w5 = wqkT.reshape(16, 128, 20, 128)                    # [j, p, strip, q]
    w5 = w5.transpose(2, 1, 0, 3)                          # [strip, p, j, q]
    order = [16, 17, 18, 19] + list(range(16))
    wqk = np.ascontiguousarray(w5[order]).astype(mmt)      # [20, 128, 16, 128]
    wv5 = wvT.reshape(16, 128, 4, 128).transpose(2, 1, 0, 3)
    wv = np.ascontiguousarray(wv5).astype(mmt)             # [4, 128, 16, 128]
    # pw[p, n, kd, q] = pwT[kd*128+p, n*512+q]
    pw6 = pwT.reshape(16, 128, 4, 512).transpose(1, 2, 0, 3)
    pw = np.ascontiguousarray(pw6).astype(mmt)             # [128, 4, 16, 512]

    bqk = np.ascontiguousarray(attn_b[pqk].reshape(20, 128).T)   # [128, 20]
    bv = np.tile(attn_b[pv][None, :], (128, 1)).astype(np.float32)
    pb = np.tile(proj_b[None, :], (128, 1)).astype(np.float32)
    m0, m1 = host_masks()
    # fast-exp additive bias tiles
    mbias = np.empty((128, 2, 1024), np.float32)
    mbias[:, 0, :] = np.where(np.concatenate([m0] * 4, axis=1) > 0, B16, -1e6)
    mbias[:, 1, :] = np.where(np.concatenate([m1] * 4, axis=1) > 0, B16, -1e6)

    in_maps = []
    for c in range(NCORES):
        b, r = c // 4, c % 4
        ids = tok_ids(r)
        xT = x[b, ids, :].T.reshape(16, 128, 512).transpose(1, 0, 2)
        xT = np.ascontiguousarray(xT).astype(mmt)          # [128, 16, 512]
        cl = cos[ids, :].T                                 # [64, 512]
        sl = sin[ids, :].T.copy()
        sl[32:] *= -1.0
        cosT2 = np.concatenate([cl, cl], axis=0).astype(mmt)
        sinT2s = np.concatenate([sl, sl], axis=0).astype(mmt)
        vA, vB = valid_tables(r)
        # vbias is indexed by the attend() LOOP slot: slots 0,1 are the
        # always-valid local slots; slot s>=2 maps to prefix slot s-2.
        vbias = np.full((128, 32), -30000.0, np.float32)
        vbias[:, 0:2] = 0.0
        vbias[:, 16:18] = 0.0
        for s in range(2, 16):
            if vB[s - 2] > 0:
                vbias[:, s] = 0.0
        for s in range(2, 8):
            if vA[s - 2] > 0:
                vbias[:, 16 + s] = 0.0
        in_maps.append({
            "xT": xT, "wqk": wqk, "wv": wv, "pw": pw,
            "bqk": bqk, "bv": bv, "pb": pb,
            "cosT2": cosT2, "sinT2s": sinT2s,
            "mbias": mbias, "vbias": vbias,
            "vones": np.ones((128, 4, 8), mmt),
        })
    return in_maps


def assemble_output(results):
    out = np.zeros((B, T, C), np.float32)
    for c in range(NCORES):
        b, r = c // 4, c % 4
        ids = tok_ids(r)
        out[b, ids, :] = results[c]["out"]
    return out


def kernel(**inputs):
    from concourse.bass_utils import run_bass_kernel_spmd

    prog = _build_program()
    in_maps = make_in_maps(**inputs)
    res = run_bass_kernel_spmd(prog["nc"], in_maps, list(range(NCORES)))
    return assemble_output(res.results)


if __name__ == "__main__":
    import reference

    inputs = {k: np.asarray(v) for k, v in reference.setup_inputs().items()}
    expected = np.asarray(reference.reference(**inputs))
    actual = kernel(**inputs)
    err = np.abs(actual - expected).max()
    rel = np.abs(actual - expected).max() / np.abs(expected).max()
    print(f"abs={err:.3e} rel={rel:.3e}")
